# revision 5
# baseline (speedup 1.0000x reference)
"""Grouped MLP (MoE expert FFN) Bass kernel for 8 Trainium2 NeuronCores.

Problem: 4096 tokens sorted by expert (8 experts, uneven counts), per-expert
GLU MLP:  h = x @ w1[g]  (-> up|gate, 2*2048 cols);  a = silu(up)*gate;
y = a @ w2[g].

Sharding: tensor-parallel over the INTER dim.  Core c owns a 256-wide slice
of INTER for ALL experts: fc1 column-slice (256 up cols + 256 gate cols per
expert), fc2 row-slice (256 rows per expert).  Every core processes every
token, so per-core work is 512-token-equivalent regardless of the expert
token distribution (perfect load balance), and each weight byte lands on
exactly one core.  Partial fc2 outputs are summed on the host.

Device program (per core), bf16 matmuls accumulated in fp32 PSUM, in
"feature-major" (transposed) space; tokens are processed in chunks of <=512
(one chunk belongs to one expert):
  hT[p]  = sum_k w1s[k,p]^T @ xT[k]       p in {pair0, pair1} x {up, gt}
  hglT   = silu(up_p) * gate_p            (ACT + DVE, PSUM->SBUF, bf16)
  yT[hb] = sum_ki w2s[ki,hb]^T @ hglT[ki] (2-term accum), cast bf16, DMA out

All DRAM<->SBUF transfers are laid out host-side so each DMA moves
[128 partitions x multi-KB contiguous lines].  Weights stream in three
0.5MB pieces per expert (up|gt pair 0, pair 1, w2) with the w2 piece
deferred one chunk, so the first matmul only waits on ~1MB of DMA.
fc2 of chunk i is emitted after fc1 of chunk i+1 (software skew) so the
PE never waits on the GLU of the chunk it just produced.  The last chunk
is capped at 128 tokens to shrink the kernel tail.
"""

import sys

try:  # concourse normally comes from the container's PYTHONPATH
    import concourse  # noqa: F401
except ImportError:  # pragma: no cover - fallback for stripped env
    for _p in (
        "/root/.axon_site",
        "/root/.axon_site/_ro/trn_rl_repo",
        "/root/.axon_site/_ro/pypackages",
        "/opt/trn_rl_repo",
    ):
        if _p not in sys.path:
            sys.path.append(_p)

from contextlib import ExitStack

import numpy as np
import ml_dtypes

BF16 = np.dtype(ml_dtypes.bfloat16)

NUM_TOKENS = 4096
HIDDEN = 1024
INTER = 2048
GROUPS = 8
N_CORES = 8

SLICE = INTER // N_CORES       # 256 inter cols/rows per core
CHUNK = 512                    # max tokens per chunk (PSUM fp32 free-dim cap)
KC = HIDDEN // 128             # 8 contraction blocks for fc1
PAIR_COLS = KC * 256           # 2048 cols per up|gt pair piece
W1_COLS = 2 * PAIR_COLS        # 4096
W2_COLS = 2 * HIDDEN           # 2048: per-ki 1024 hid cols
WC_COLS = W1_COLS + W2_COLS    # 6144


def _chunks_from_counts(counts):
    """Split each expert's token range into near-equal chunks of <= CHUNK."""
    chunks = []  # (expert, token_offset, n)
    off = 0
    for g in range(GROUPS):
        cnt = int(counts[g])
        if cnt <= 0:
            continue
        parts = -(-cnt // CHUNK)
        base, rem = divmod(cnt, parts)
        for i in range(parts):
            n = base + (1 if i < rem else 0)
            chunks.append((g, off, n))
            off += n
    # small last chunk -> short kernel tail (fc2+store of the final chunk
    # cannot overlap anything)
    if chunks and chunks[-1][2] > 256:
        g, off, n = chunks[-1]
        chunks[-1] = (g, off, n - 128)
        chunks.append((g, off + n - 128, 128))
    return chunks


_PROGRAM_CACHE: dict = {}


def _build_program(key):
    """Build + compile the single-core Bass program (same NEFF on all cores).

    key = tuple of (expert, n_tokens) per chunk, in token order.
    """
    import concourse.bass as bass  # noqa: F401
    import concourse.mybir as mybir
    import concourse.tile as tile
    from concourse import bacc

    f32 = mybir.dt.float32
    bf16 = mybir.dt.bfloat16
    silu = mybir.ActivationFunctionType.Silu

    T = sum(n for _, n in key)

    nc = bacc.Bacc("TRN2", target_bir_lowering=False, debug=False)

    x_d = nc.dram_tensor("xc", [128, KC * T], bf16, kind="ExternalInput").ap()
    w_d = nc.dram_tensor("wc", [GROUPS, 128, WC_COLS], bf16, kind="ExternalInput").ap()
    y_d = nc.dram_tensor("yc", [128, 8 * T], bf16, kind="ExternalOutput").ap()

    with tile.TileContext(nc) as tc, ExitStack() as ctx:
        xp = ctx.enter_context(tc.tile_pool(name="x", bufs=6))
        wp = ctx.enter_context(tc.tile_pool(name="w", bufs=5))
        hp = ctx.enter_context(tc.tile_pool(name="hgl", bufs=3))
        yp = ctx.enter_context(tc.tile_pool(name="y", bufs=3))
        tp = ctx.enter_context(tc.tile_pool(name="tmp", bufs=4))
        p1 = ctx.enter_context(tc.tile_pool(name="p1", bufs=5, space="PSUM"))
        p2 = ctx.enter_context(tc.tile_pool(name="p2", bufs=3, space="PSUM"))

        wt = {}          # expert -> SBUF weight tile
        p2_pending = []  # experts whose w2 piece DMA is deferred

        def flush_p2():
            while p2_pending:
                g = p2_pending.pop(0)
                nc.sync.dma_start(
                    out=wt[g][:, W1_COLS:WC_COLS], in_=w_d[g][:, W1_COLS:WC_COLS]
                )

        def emit_fc2(g, hgl, off, n):
            w = wt[g]
            y_sb = yp.tile([128, 8 * n], bf16, tag="y")
            for hb in range(8):
                py = p2.tile([128, n], f32, tag="p2")
                nc.tensor.matmul(
                    py,
                    w[:, W1_COLS + hb * 128 : W1_COLS + hb * 128 + 128],
                    hgl[:, :n],
                    start=True,
                    stop=False,
                )
                nc.tensor.matmul(
                    py,
                    w[:, W1_COLS + HIDDEN + hb * 128 : W1_COLS + HIDDEN + hb * 128 + 128],
                    hgl[:, n : 2 * n],
                    start=False,
                    stop=True,
                )
                dst = y_sb[:, hb * n : (hb + 1) * n]
                if hb % 2 == 0:
                    nc.scalar.copy(dst, py)
                else:
                    nc.vector.tensor_copy(dst, py)
            # outputs go on the ACT HWDGE ring so a y store blocked on its
            # casts can't head-of-line-block input DMAs on the sync ring
            nc.scalar.dma_start(out=y_d[:, 8 * off : 8 * (off + n)], in_=y_sb)

        pending = None  # (expert, hgl tile, token_offset, n) awaiting fc2
        off = 0
        for g, n in key:
            xt = xp.tile([128, KC * n], bf16, tag="x")
            nc.sync.dma_start(out=xt, in_=x_d[:, KC * off : KC * (off + n)])
            if g not in wt:
                w = wp.tile([128, WC_COLS], bf16, tag="w")
                nc.sync.dma_start(out=w[:, :PAIR_COLS], in_=w_d[g][:, :PAIR_COLS])
                nc.sync.dma_start(
                    out=w[:, PAIR_COLS:W1_COLS], in_=w_d[g][:, PAIR_COLS:W1_COLS]
                )
                wt[g] = w
                deferred = True
            else:
                deferred = False

            # fc1: pair p in {0, 1}; piece p holds k-major [up_p | gt_p]
            ps = {}
            for p in (0, 1):
                for half in (0, 1):  # 0: up, 1: gate
                    acc = p1.tile([128, n], f32, tag="p1")
                    for k in range(KC):
                        base = p * PAIR_COLS + k * 256 + half * 128
                        nc.tensor.matmul(
                            acc,
                            wt[g][:, base : base + 128],
                            xt[:, k * n : (k + 1) * n],
                            start=(k == 0),
                            stop=(k == KC - 1),
                        )
                    ps[(p, half)] = acc

            hgl = hp.tile([128, 2 * n], bf16, tag="h")
            for p in range(2):
                tmp = tp.tile([128, n], f32, tag="t")
                nc.scalar.activation(tmp, ps[(p, 0)], silu)
                nc.vector.tensor_mul(hgl[:, p * n : (p + 1) * n], tmp, ps[(p, 1)])

            flush_p2()
            if deferred:
                p2_pending.append(g)
            if pending is not None:
                emit_fc2(*pending)
            pending = (g, hgl, off, n)
            off += n

        flush_p2()
        emit_fc2(*pending)

    nc.compile()
    return nc


def _get_program(key):
    if key not in _PROGRAM_CACHE:
        _PROGRAM_CACHE[key] = _build_program(key)
    return _PROGRAM_CACHE[key]


def _prep_x(x, chunks, T):
    """[T, 1024] fp32 -> [128, 8*T] bf16, chunk-major k-blocked layout."""
    xb = x.astype(BF16)
    X = np.empty((128, KC * T), BF16)
    for _, off, n in chunks:
        seg = xb[off : off + n].T  # [1024, n]
        X[:, KC * off : KC * (off + n)] = (
            seg.reshape(KC, 128, n).transpose(1, 0, 2).reshape(128, KC * n)
        )
    return X


def _prep_weights(w1b, w2b, c):
    """Per-core slices of w1/w2 (already bf16) -> [8, 128, 6144].

    cols [p*2048 + k*256 + half*128 : +128] = w1 block (pair p, k, up/gt)
    cols [4096 + ki*1024 : +1024]           = w2 block ki
    """
    wc = np.empty((GROUPS, 128, WC_COLS), BF16)
    lo = c * SLICE
    for g in range(GROUPS):
        for p in range(2):
            u = w1b[g][:, lo + p * 128 : lo + p * 128 + 128]
            gt = w1b[g][:, INTER + lo + p * 128 : INTER + lo + p * 128 + 128]
            sl = np.concatenate([u, gt], 1)  # [1024, 256]
            wc[g, :, p * PAIR_COLS : (p + 1) * PAIR_COLS] = (
                sl.reshape(KC, 128, 256).transpose(1, 0, 2).reshape(128, PAIR_COLS)
            )
        w2s = w2b[g][lo : lo + SLICE]  # [256, 1024]
        wc[g, :, W1_COLS:] = (
            w2s.reshape(2, 128, HIDDEN).transpose(1, 0, 2).reshape(128, W2_COLS)
        )
    return wc


_LAST_RESULTS = {}  # exposed for test.py (exec time, trace paths)


def kernel(permuted_tokens, tokens_per_expert, w1, w2, _trace=False):
    from concourse.bass_utils import run_bass_kernel_spmd

    x = np.asarray(permuted_tokens, np.float32)
    counts = np.asarray(tokens_per_expert, np.int64)
    w1 = np.asarray(w1, np.float32)
    w2 = np.asarray(w2, np.float32)

    chunks = _chunks_from_counts(counts)
    T = sum(n for _, _, n in chunks)
    key = tuple((g, n) for g, _, n in chunks)

    nc = _get_program(key)

    X = _prep_x(x, chunks, T)
    w1b = w1.astype(BF16)
    w2b = w2.astype(BF16)
    in_maps = [{"xc": X, "wc": _prep_weights(w1b, w2b, c)} for c in range(N_CORES)]

    kwargs = {}
    if _trace:
        kwargs = dict(trace=True, trace_cores=list(range(N_CORES)))
    res = run_bass_kernel_spmd(nc, in_maps, core_ids=list(range(N_CORES)), **kwargs)
    _LAST_RESULTS["res"] = res

    acc = np.zeros((128, 8 * T), np.float32)
    for c in range(N_CORES):
        acc += np.asarray(res.results[c]["yc"]).astype(np.float32)

    out = np.zeros((x.shape[0], HIDDEN), np.float32)
    for _, off, n in chunks:
        seg = acc[:, 8 * off : 8 * (off + n)].reshape(128, 8, n)
        out[off : off + n] = seg.transpose(2, 1, 0).reshape(n, HIDDEN)
    return out
